# revision 40
# baseline (speedup 1.0000x reference)
"""Trainium2 Bass kernel for nn_Attention_78554951844258.

Dense 12-head attention block: qkv = x@Wqkv+b; RoPE(q,k); softmax(q k^T/sqrt(d)) v; proj.

Sharding: data-parallel over batch — each of the 8 NeuronCores computes one
batch element end-to-end (no collectives).

Algebraic restructuring (host-side, exact, O(weights)):
  * The reference applies RoPE with seq_dim=1 on [b,h,n,d], so cos/sin depend
    only on (head, dim) — RoPE is a position-independent per-head 64x64 linear
    map M_h that folds into the q/k columns of w_qkv (and biases).
  * The softmax scale 1/sqrt(d) folds into the q weights.
  * The v bias and proj bias fold into a single output bias
    b_out = b_v @ w_proj + b_proj, because softmax rows sum to 1.
  * Softmax max-subtraction is skipped: folded scores are bounded (|S| < ~3),
    exp is safe in fp32 and the result is mathematically identical.

Numerics: the q/k projections run in fp8e4 (e4m3) with DoubleRow perf mode
(256-row contraction per matmul, 0.5 cycles/row): weights are scaled x256
host-side into the e4m3 normal range and exp() applies the 1/65536
correction via its scale operand. The v/attention-value path stays bf16
(output magnitude shrinks like the softmax average, so value-path
quantization noise lands ~fully in the relative error; q/k noise is damped
through the scores). Measured rel l2 err ~1.0e-2 (vs 1.9e-3 all-bf16).

Schedule: engineered for Tensor-engine continuity (TRN2 PE runs at 1.2 GHz
until ~3us of continuous execution, 2.4 GHz after — any gap resets it):
  * PSUM split: tag "pv" (2 x 4KB slots: AV accumulators [65,1024], also
    v/proj psum) + tag "st" (2 x 4KB slots: score tiles, qk psum) so the
    next head-pair's score matmuls never wait on the previous pair's
    normalization.
  * Prologue: all 12 q/k projections in one ramped PE burst, interleaved
    with head-pair 0's score tiles (primes the ACT exp stream) and the v
    tiles (paced behind their DMA); weights stream in as row-halves in
    consumption order.
  * Steady state: per head pair only scores+AV remain; AV matmuls trail the
    score stream by 2 j-tiles so the pv-slot handoff has ~4us of slack and
    the PE phase-locks into a gapless [st,st,av,av] cadence at 2.4 GHz,
    paced by ACT's exp chain (~1.08us per [128,1024] tile).
  * Normalization: free the PSUM accumulators immediately (one bf16 copy
    per head incl. the colsum row), then entirely out of SBUF: sum rows
    regathered to [128,16] via direct SBUF->SBUF partition-scatter DMAs
    (lane-parallel ~0.25us reciprocal vs 6.5us single-lane), scattered to
    DRAM, broadcast back via zero-partition-stride DRAM APs, DVE multiply;
    odd head staged to its ovT partition range by an SBUF->SBUF DMA.
  * Projection in two passes: e=0..4 partials (4-deep across both PSUM
    pools) overlap the last head pair's normalization chain; the e=5 term,
    bias and chunked stores finish each i-tile.
"""
import numpy as np

NUM_HEADS = 12
E = 768
D = 64
B = 8
N = 1024
HALF = D // 2


def _ensure_axon_hooks():
    """The NTFF profile hook registry module may be missing in a fresh
    container; (re)create it so trace=True profiling degrades gracefully."""
    try:
        import antenv.axon_hooks  # noqa: F401
        return
    except ImportError:
        pass
    try:
        import antenv
        import os
        p = os.path.join(os.path.dirname(antenv.__file__), "axon_hooks.py")
        with open(p, "w") as f:
            f.write(
                "_hook = None\n\n"
                "def set_axon_ntff_profile_hook(hook):\n"
                "    global _hook\n    _hook = hook\n\n"
                "def get_axon_ntff_profile_hook():\n"
                "    return _hook\n")
    except Exception:
        pass


_ensure_axon_hooks()


# ---------------------------------------------------------------- host math
def _rope_matrix():
    """M[h, x, d]: rope(q)[x] = sum_d M[h, x, d] * q[d] (float64)."""
    inv_freq = 1.0 / (10000.0 ** (np.arange(0, D, 2, dtype=np.float64) / D))
    t = np.arange(NUM_HEADS, dtype=np.float64)
    emb = np.concatenate([t[:, None] * inv_freq[None, :]] * 2, axis=-1)  # [H, D]
    cos, sin = np.cos(emb), np.sin(emb)
    M = np.zeros((NUM_HEADS, D, D))
    for h in range(NUM_HEADS):
        for d in range(D):
            M[h, d, d] = cos[h, d]
            if d < HALF:
                M[h, d, d + HALF] = -sin[h, d]
            else:
                M[h, d, d - HALF] = sin[h, d]
    return M


def _prep_weights(w_qkv, b_qkv, w_proj, b_proj):
    w = w_qkv.astype(np.float64)
    b = b_qkv.astype(np.float64)
    M = _rope_matrix()
    scale = float(D) ** (-0.5)
    w_q = w[:, 0:E].reshape(E, NUM_HEADS, D)
    w_k = w[:, E:2 * E].reshape(E, NUM_HEADS, D)
    b_q = b[0:E].reshape(NUM_HEADS, D)
    b_k = b[E:2 * E].reshape(NUM_HEADS, D)
    w_q2 = np.einsum('ehd,hxd->ehx', w_q, M) * scale
    b_q2 = np.einsum('hd,hxd->hx', b_q, M) * scale
    w_k2 = np.einsum('ehd,hxd->ehx', w_k, M)
    b_k2 = np.einsum('hd,hxd->hx', b_k, M)
    w_qk = np.ascontiguousarray(
        np.concatenate([w_q2.reshape(E, E), w_k2.reshape(E, E)], axis=1),
        dtype=np.float32)                                     # [E, 2E]
    b_qk = np.concatenate([b_q2.reshape(E), b_k2.reshape(E)]).astype(np.float32)
    w_v = np.ascontiguousarray(w[:, 2 * E:3 * E], dtype=np.float32)
    b_out = (b[2 * E:3 * E] @ w_proj.astype(np.float64)
             + b_proj.astype(np.float64)).astype(np.float32)
    return w_qk, b_qk, w_v, b_out


# ---------------------------------------------------------------- waitfix
def _split_excess_waits(nc):
    """walrus in this container rejects >4 sync waits per instruction (and
    fewer on Drain/SP-NoOp paths). Split overflow waits onto preceding
    same-engine 1-wait NOPs — semantically identical (sequencer blocks in
    order)."""
    import concourse.mybir as mybir
    import bass_rust
    counter = [0]

    def make_nop(engine):
        counter[0] += 1
        nop = bass_rust.InstNoOp(name=f"I-waitfix-{counter[0]}", ins=[], outs=[])
        nop.engine = engine
        return nop

    for fn in nc.m.functions:
        for bb in fn.blocks:
            insts = bb.instructions
            out = []
            changed = False
            for inst in insts:
                si = inst.sync_info
                waits = list(si.on_wait) if si is not None else []
                tn = type(inst).__name__
                keep = 0 if tn == "InstDrain" else 1
                if len(waits) > keep:
                    for w in waits[:len(waits) - keep]:
                        nop = make_nop(inst.engine)
                        nop.sync_info = mybir.SyncInfo(on_wait=[w], on_update=[])
                        out.append(nop)
                    inst.sync_info = mybir.SyncInfo(
                        on_wait=waits[len(waits) - keep:],
                        on_update=list(si.on_update))
                    changed = True
                out.append(inst)
            if changed:
                bb.instructions = out


# ---------------------------------------------------------------- device IR
_NC_CACHE = []


def _build_nc(waitfix=True):
    import concourse.bass as bass
    import concourse.mybir as mybir
    from concourse.tile import TileContext

    dt = mybir.dt
    f32 = dt.float32
    bf16 = dt.bfloat16
    AF = mybir.ActivationFunctionType

    nc = bass.Bass(target_bir_lowering=False)
    f8 = dt.float8e4
    xT_d = nc.dram_tensor("xT", [E, N], bf16, kind="ExternalInput")
    xT8_d = nc.dram_tensor("xT8", [3, 128, 2 * N], f8, kind="ExternalInput")
    wqk8_d = nc.dram_tensor("w_qk8", [3, 128, 4 * E], f8, kind="ExternalInput")
    bqk_d = nc.dram_tensor("b_qk", [2 * E], f32, kind="ExternalInput")
    wv_d = nc.dram_tensor("w_v", [E, E], bf16, kind="ExternalInput")
    wp_d = nc.dram_tensor("w_proj", [E, E], bf16, kind="ExternalInput")
    bo_d = nc.dram_tensor("b_out", [E], f32, kind="ExternalInput")
    y_d = nc.dram_tensor("y", [N, E], f32, kind="ExternalOutput")

    ET = E // 128          # 6 e-tiles
    IT = N // 128          # 8 i/j-tiles
    HP = NUM_HEADS // 2    # 6 head pairs

    with TileContext(nc) as tc:
        with (
            tc.tile_pool(name="load", bufs=1) as pl,
            tc.tile_pool(name="qtk", bufs=12) as pqt,
            tc.tile_pool(name="pt", bufs=16) as ppt,
            tc.tile_pool(name="vaug", bufs=1) as pva,
            tc.tile_pool(name="ov", bufs=1) as pov,
            tc.tile_pool(name="raw", bufs=4) as prw,
            tc.tile_pool(name="rs", bufs=2) as prs,
            tc.tile_pool(name="rb", bufs=4) as prb,
            tc.tile_pool(name="tmp", bufs=2) as ptm,
            tc.tile_pool(name="yst", bufs=2) as pys,
            tc.tile_pool(name="psv", bufs=2, space="PSUM") as psv,
            tc.tile_pool(name="psst", bufs=2, space="PSUM") as pss,
            tc.tile_pool(name="dscr", bufs=4, space="DRAM") as pdram,
        ):
            # ---- input loads (xT/wqk first: they gate the first PE work)
            xT = [pl.tile([128, N], bf16, tag=f"xT{e}", name=f"xT{e}")
                  for e in range(ET)]
            xT8 = [pl.tile([128, 2 * N], f8, tag=f"xT8_{p}", name=f"xT8_{p}")
                   for p in range(3)]
            wqk8 = [pl.tile([128, 4 * E], f8, tag=f"wqk8_{p}",
                            name=f"wqk8_{p}") for p in range(3)]
            wv = [pl.tile([128, E], bf16, tag=f"wv{e}", name=f"wv{e}")
                  for e in range(ET)]
            wp = [pl.tile([128, E], bf16, tag=f"wp{e}", name=f"wp{e}")
                  for e in range(ET)]
            # all weight/activation loads as partition-row halves: full-width
            # lines (good DMA efficiency) on 2x the queues, in consumption
            # order (xT+wqk gate the first matmuls, wv the v phase, wp proj)
            def load_halves(tiles, dram, e):
                for h in range(2):
                    rs_ = slice(e * 128 + h * 64, e * 128 + (h + 1) * 64)
                    nc.sync.dma_start(out=tiles[e][h * 64:(h + 1) * 64, :],
                                      in_=dram[rs_, :])
            # fp8 qk operands first (they gate the first matmuls), then the
            # bf16 xT (v path); all as row-halves across queues
            for p in range(3):
                for h in range(2):
                    rs_ = slice(h * 64, (h + 1) * 64)
                    nc.sync.dma_start(out=wqk8[p][rs_, :],
                                      in_=wqk8_d[p, rs_, :])
                    nc.sync.dma_start(out=xT8[p][rs_, :],
                                      in_=xT8_d[p, rs_, :])
            for e in range(ET):
                load_halves(xT, xT_d, e)
            bq = pl.tile([128, 12], f32, tag="bq")
            nc.sync.dma_start(out=bq, in_=bqk_d[:].rearrange("(t p) -> p t", p=128))
            for e in range(ET):
                load_halves(wv, wv_d, e)
            for e in range(ET):
                load_halves(wp, wp_d, e)
            bo = pl.tile([128, E], f32, tag="bo")
            nc.sync.dma_start(
                out=bo,
                in_=bass.AP(tensor=bo_d[:].tensor, offset=bo_d[:].offset,
                            ap=[[0, 128], [1, E]]))

            v_aug = [pva.tile([128, NUM_HEADS * (D + 1)], bf16, tag=f"vaug{i}",
                              name=f"vaug{i}") for i in range(IT)]
            ovT = [pov.tile([128, N], bf16, tag=f"ovT{e}", name=f"ovT{e}")
                   for e in range(ET)]

            # ---------------- emit helpers
            def emit_qk(ct):
                """q^T (or k^T) for column-tile ct -> [128, N] bf16 SBUF.

                fp8 DoubleRow: each matmul contracts a 256-row e-pair (two
                k-subtiles along the free dim), 0.5 cycles/row. Values are
                scaled by 256 (folded into weights host-side; exp applies
                1/65536), keeping fp8 quantization in the e4m3 normal range.
                """
                pq = pss.tile([128, N], f32, tag="st", name=f"pq{ct}")
                for ih in range(2):
                    for p in range(3):
                        wv3 = wqk8[p].rearrange("q (t c) -> q t c", t=2)
                        xv3 = xT8[p].rearrange("q (t c) -> q t c", t=2)
                        nc.tensor.matmul(
                            pq[:, ih * 512:(ih + 1) * 512],
                            wv3[:, :, ct * 128:(ct + 1) * 128],
                            xv3[:, :, ih * 512:(ih + 1) * 512],
                            start=(p == 0), stop=(p == 2),
                            perf_mode=mybir.MatmulPerfMode.DoubleRow)
                dst = pqt.tile([128, N], bf16, tag="qt", name=f"qkT{ct}")
                nc.vector.tensor_scalar_add(dst, pq, bq[:, ct:ct + 1])
                return dst

            def emit_v(it):
                """v rows for i-tile it -> v_aug[it] (bf16, ones cols)."""
                pv = psv.tile([128, E], f32, tag="pv", name=f"pv_{it}")
                for (n0, nw) in ((0, 512), (512, 256)):
                    for e in range(ET):
                        nc.tensor.matmul(
                            pv[:, n0:n0 + nw],
                            xT[e][:, it * 128:(it + 1) * 128],
                            wv[e][:, n0:n0 + nw],
                            start=(e == 0), stop=(e == ET - 1))
                va = v_aug[it]
                out_v = va.rearrange("p (h c) -> p h c", c=65)[:, :, 0:64]
                in_v = pv.rearrange("p (h c) -> p h c", c=64)
                nc.vector.tensor_copy(out=out_v, in_=in_v)
                ones_cols = va.rearrange("p (h c) -> p h c", c=65)[:, :, 64:65]
                bq12 = bq[:, 0:12].rearrange("p (a b) -> p a b", b=1)
                nc.vector.tensor_scalar(
                    ones_cols, bq12, 0.0, 1.0,
                    mybir.AluOpType.mult, mybir.AluOpType.add)

            def emit_st(hp, jt, h, qt, kt):
                """scores^T tile for (head-pair hp, j-tile jt, head h) + exp."""
                js = slice(jt * 128, (jt + 1) * 128)
                hs = slice(h * 64, h * 64 + 64)
                st = pss.tile([128, N], f32, tag="st", name=f"st{hp}_{jt}_{h}")
                for ih in range(2):
                    isl = slice(ih * 512, (ih + 1) * 512)
                    nc.tensor.matmul(st[:, isl], kt[hs, js], qt[hs, isl])
                pt = ppt.tile([128, N], bf16, tag="pT", name=f"pT{hp}_{jt}_{h}")
                nc.scalar.activation(out=pt, in_=st, func=AF.Exp,
                                     scale=1.0 / 65536.0)
                return pt

            def emit_av(hp, jt, pts, pv0, pv1):
                h0, h1 = 2 * hp, 2 * hp + 1
                pt0, pt1 = pts
                for (pv, hh, pt) in ((pv0, h0, pt0), (pv1, h1, pt1)):
                    for ih in range(2):
                        isl = slice(ih * 512, (ih + 1) * 512)
                        nc.tensor.matmul(
                            pv[:, isl], v_aug[jt][:, hh * 65:hh * 65 + 65],
                            pt[:, isl], start=(jt == 0), stop=(jt == IT - 1))

            def emit_norm(hp, pv0, pv1):
                """ovT[hp] = pv / colsums.

                pv PSUM slots gate the NEXT head-pair's AV matmuls, so free
                them ASAP: one bf16 copy per head to SBUF (raw out + sums),
                then normalize entirely out of SBUF. The [1,N] colsum rows
                are single-lane; a direct DVE reciprocal is 6-pass (~6.5us),
                so round-trip through DRAM to regather them as [128,16]
                (lane-parallel, ~0.25us), scatter back, broadcast-in via
                zero-partition-stride DRAM APs, DVE multiply."""
                raw0 = prw.tile([65, N], bf16, tag="raw", name=f"raw0_{hp}")
                raw1 = prw.tile([65, N], bf16, tag="raw", name=f"raw1_{hp}")
                nc.vector.tensor_copy(out=raw0, in_=pv0)
                if hp == HP - 1:
                    # last pair: ACT is idle; run the second copy there in
                    # parallel so the projection tail starts sooner
                    nc.scalar.activation(out=raw1, in_=pv1, func=AF.Copy)
                else:
                    nc.vector.tensor_copy(out=raw1, in_=pv1)
                # lane-parallel reciprocal: gather both [1,N] sum rows into
                # [128,16] via direct SBUF->SBUF partition-scatter DMAs
                rg = prs.tile([128, 16], bf16, tag="rg", name=f"rg{hp}")
                nc.sync.dma_start(
                    out=rg[0:64, :],
                    in_=raw0[64:65, :].rearrange("p (a b) -> p a b", b=16))
                nc.sync.dma_start(
                    out=rg[64:128, :],
                    in_=raw1[64:65, :].rearrange("p (a b) -> p a b", b=16))
                rgr = prs.tile([128, 16], bf16, tag="rgr", name=f"rgr{hp}")
                with nc.allow_low_precision(reason="softmax denominators"):
                    nc.vector.reciprocal(out=rgr, in_=rg)
                dr = pdram.tile([1, 2 * N], bf16, tag="dr", name=f"dr{hp}")
                nc.sync.dma_start(
                    out=bass.AP(tensor=dr.tensor, offset=dr.offset,
                                ap=[[16, 128], [1, 16]]),
                    in_=rgr)
                rb0 = prb.tile([64, N], bf16, tag="rb", name=f"rb0_{hp}")
                rb1 = prb.tile([64, N], bf16, tag="rb", name=f"rb1_{hp}")
                nc.sync.dma_start(
                    out=rb0, in_=bass.AP(tensor=dr.tensor, offset=dr.offset,
                                         ap=[[0, 64], [1, N]]))
                nc.sync.dma_start(
                    out=rb1, in_=bass.AP(tensor=dr.tensor,
                                         offset=dr.offset + N,
                                         ap=[[0, 64], [1, N]]))
                tmp1 = ptm.tile([64, N], bf16, tag="tmp", name=f"tmp{hp}")
                for c in (slice(0, 512), slice(512, N)):
                    nc.vector.tensor_mul(ovT[hp][0:64, c], raw0[0:64, c],
                                         rb0[:, c])
                    nc.vector.tensor_mul(tmp1[:, c], raw1[0:64, c],
                                         rb1[:, c])
                    nc.sync.dma_start(out=ovT[hp][64:128, c], in_=tmp1[:, c])

            # ---------------- prologue: all 12 qk projections in one ramped
            # PE burst, with hp0's score tiles interleaved to prime ACT
            qts, kts = {}, {}
            pts = {}

            def emit_qk_pair(ct_a, ct_b):
                """First q/k pair with e-interleaved accumulation: matmuls
                track the (wqk[e], xT[e]) DMA arrival order, halving the
                load-paced stalls at kernel start."""
                pqa = pss.tile([128, N], f32, tag="st", name=f"pq{ct_a}")
                pqb = pss.tile([128, N], f32, tag="st", name=f"pq{ct_b}")
                for p in range(3):
                    wv3 = wqk8[p].rearrange("q (t c) -> q t c", t=2)
                    xv3 = xT8[p].rearrange("q (t c) -> q t c", t=2)
                    for (pq, ct) in ((pqa, ct_a), (pqb, ct_b)):
                        for ih in range(2):
                            nc.tensor.matmul(
                                pq[:, ih * 512:(ih + 1) * 512],
                                wv3[:, :, ct * 128:(ct + 1) * 128],
                                xv3[:, :, ih * 512:(ih + 1) * 512],
                                start=(p == 0), stop=(p == 2),
                                perf_mode=mybir.MatmulPerfMode.DoubleRow)
                outs = []
                for (pq, ct) in ((pqa, ct_a), (pqb, ct_b)):
                    dst = pqt.tile([128, N], bf16, tag="qt", name=f"qkT{ct}")
                    nc.vector.tensor_scalar_add(dst, pq, bq[:, ct:ct + 1])
                    outs.append(dst)
                return outs

            for hp0 in range(HP):
                if hp0 == 0:
                    qts[0], kts[0] = emit_qk_pair(0, HP)
                    continue
                qts[hp0] = emit_qk(hp0)
                kts[hp0] = emit_qk(HP + hp0)
                if hp0 >= 1:
                    jt = hp0 - 1
                    pts[(jt, 0)] = emit_st(0, jt, 0, qts[0], kts[0])
                    pts[(jt, 1)] = emit_st(0, jt, 1, qts[0], kts[0])
                if 2 <= hp0 <= 5:
                    emit_v(2 * (hp0 - 2))
                    emit_v(2 * (hp0 - 2) + 1)
            # hp0's remaining score tiles + v tail + AV catch-up
            pv0 = psv.tile([65, N], f32, tag="pv", name="pv0_0")
            pv1 = psv.tile([65, N], f32, tag="pv", name="pv1_0")
            pts[(5, 0)] = emit_st(0, 5, 0, qts[0], kts[0])
            pts[(5, 1)] = emit_st(0, 5, 1, qts[0], kts[0])
            emit_av(0, 0, (pts.pop((0, 0)), pts.pop((0, 1))), pv0, pv1)
            emit_av(0, 1, (pts.pop((1, 0)), pts.pop((1, 1))), pv0, pv1)
            pts[(6, 0)] = emit_st(0, 6, 0, qts[0], kts[0])
            pts[(6, 1)] = emit_st(0, 6, 1, qts[0], kts[0])
            emit_av(0, 2, (pts.pop((2, 0)), pts.pop((2, 1))), pv0, pv1)
            emit_av(0, 3, (pts.pop((3, 0)), pts.pop((3, 1))), pv0, pv1)
            pts[(7, 0)] = emit_st(0, 7, 0, qts[0], kts[0])
            pts[(7, 1)] = emit_st(0, 7, 1, qts[0], kts[0])
            for jt in range(4, IT):
                emit_av(0, jt, (pts.pop((jt, 0)), pts.pop((jt, 1))), pv0, pv1)
            emit_norm(0, pv0, pv1)

            # ---------------- steady-state head pairs 1..5 (scores + AV only)
            # AVs trail the score stream by 2 j-tiles: the pv-slot handoff at
            # each head-pair boundary (previous pair's raw copies) then has
            # ~4us of slack, so the PE phase-locks into the gapless
            # [st,st,av,av] pattern instead of the 1.2GHz fragmented mode
            for hp in range(1, HP):
                qt, kt = qts[hp], kts[hp]
                pv0 = psv.tile([65, N], f32, tag="pv", name=f"pv0_{hp}")
                pv1 = psv.tile([65, N], f32, tag="pv", name=f"pv1_{hp}")
                lag = {}
                for jt in range(IT):
                    if jt >= 2:
                        emit_av(hp, jt - 2, lag.pop(jt - 2), pv0, pv1)
                    lag[jt] = (emit_st(hp, jt, 0, qt, kt),
                               emit_st(hp, jt, 1, qt, kt))
                emit_av(hp, IT - 2, lag.pop(IT - 2), pv0, pv1)
                emit_av(hp, IT - 1, lag.pop(IT - 1), pv0, pv1)
                emit_norm(hp, pv0, pv1)

            # ---------------- projection: y = ovT^T @ w_proj + b_out
            # two passes: e=0..4 partials (ready after hp4) keep the PE busy
            # while hp5's normalization chain completes; e=5 finishes each
            pyts = {}

            def emit_proj_partial(it):
                pool, tag = (psv, "pv") if it % 2 == 0 else (pss, "st")
                isl = slice(it * 128, (it + 1) * 128)
                pyt = pool.tile([128, E], f32, tag=tag, name=f"py_{it}")
                for (n0, nw) in ((0, 512), (512, 256)):
                    for e in range(ET - 1):
                        nc.tensor.matmul(
                            pyt[:, n0:n0 + nw],
                            ovT[e][:, isl],
                            wp[e][:, n0:n0 + nw],
                            start=(e == 0), stop=False)
                pyts[it] = pyt

            def emit_proj_finish(it):
                isl = slice(it * 128, (it + 1) * 128)
                pyt = pyts.pop(it)
                for (n0, nw) in ((0, 512), (512, 256)):
                    nc.tensor.matmul(
                        pyt[:, n0:n0 + nw],
                        ovT[ET - 1][:, isl],
                        wp[ET - 1][:, n0:n0 + nw],
                        start=False, stop=True)
                ysb = pys.tile([128, E], f32, tag="y", name=f"y{it}")
                nc.vector.tensor_add(ysb, pyt, bo)
                nc.sync.dma_start(out=y_d[isl, 0:512], in_=ysb[:, 0:512])
                nc.sync.dma_start(out=y_d[isl, 512:E], in_=ysb[:, 512:E])

            for it in range(4):
                emit_proj_partial(it)
            for it in range(4):
                emit_proj_finish(it)
                emit_proj_partial(it + 4)
            for it in range(4, IT):
                emit_proj_finish(it)

    if waitfix:
        _split_excess_waits(nc)
    return nc


def _get_nc():
    if not _NC_CACHE:
        _NC_CACHE.append(_build_nc())
    return _NC_CACHE[0]


# ---------------------------------------------------------------- entry point
def kernel(x, w_qkv, b_qkv, w_proj, b_proj, _trace=False):
    from concourse.bass_utils import run_bass_kernel_spmd

    import ml_dtypes
    bf16 = ml_dtypes.bfloat16
    x = np.asarray(x)
    w_qk, b_qk, w_v, b_out = _prep_weights(
        np.asarray(w_qkv), np.asarray(b_qkv), np.asarray(w_proj),
        np.asarray(b_proj))
    f8 = ml_dtypes.float8_e4m3
    SW = 256.0

    def to_f8(a):
        return np.clip(a, -240.0, 240.0).astype(f8)

    def dr_pairs(a, free):
        # [E, free] -> [3, 128, 2*free] with the two 128-row k-subtiles of
        # each 256-row pair concatenated along the free dim (DoubleRow lhs/rhs)
        return np.ascontiguousarray(
            a.reshape(3, 2, 128, free).transpose(0, 2, 1, 3).reshape(
                3, 128, 2 * free))

    w_qk8 = dr_pairs(to_f8(w_qk * SW), 2 * E)
    w_v16 = w_v.astype(bf16)
    w_proj16 = np.ascontiguousarray(np.asarray(w_proj)).astype(bf16)

    in_maps = []
    for b in range(B):
        xTb = np.ascontiguousarray(x[b].T)
        in_maps.append({
            "xT": xTb.astype(bf16),
            "xT8": dr_pairs(to_f8(xTb), N),
            "w_qk8": w_qk8,
            "b_qk": b_qk * np.float32(SW),
            "w_v": w_v16,
            "w_proj": w_proj16,
            "b_out": b_out,
        })

    nc = _get_nc()
    res = run_bass_kernel_spmd(nc, in_maps, core_ids=list(range(B)),
                               trace=_trace)
    out = np.stack([res.results[b]["y"] for b in range(B)]).astype(np.float32)
    if _trace:
        return out, res
    return out


# revision 42
# speedup vs baseline: 1.1711x; 1.1711x over previous
"""Trainium2 Bass kernel for nn_Attention_78554951844258.

Dense 12-head attention block: qkv = x@Wqkv+b; RoPE(q,k); softmax(q k^T/sqrt(d)) v; proj.

Sharding: data-parallel over batch — each of the 8 NeuronCores computes one
batch element end-to-end (no collectives).

Algebraic restructuring (host-side, exact, O(weights)):
  * The reference applies RoPE with seq_dim=1 on [b,h,n,d], so cos/sin depend
    only on (head, dim) — RoPE is a position-independent per-head 64x64 linear
    map M_h that folds into the q/k columns of w_qkv (and biases).
  * The softmax scale 1/sqrt(d) folds into the q weights.
  * The v bias and proj bias fold into a single output bias
    b_out = b_v @ w_proj + b_proj, because softmax rows sum to 1.
  * Softmax max-subtraction is skipped: folded scores are bounded (|S| < ~3),
    exp is safe in fp32 and the result is mathematically identical.

Numerics: the q/k projections run in fp8e4 (e4m3) with DoubleRow perf mode
(256-row contraction per matmul, 0.5 cycles/row): weights are scaled x256
host-side into the e4m3 normal range and exp() applies the 1/65536
correction via its scale operand. The v/attention-value path stays bf16
(output magnitude shrinks like the softmax average, so value-path
quantization noise lands ~fully in the relative error; q/k noise is damped
through the scores). Measured rel l2 err ~1.0e-2 (vs 1.9e-3 all-bf16).

Schedule: engineered for Tensor-engine continuity (TRN2 PE runs at 1.2 GHz
until ~3us of continuous execution, 2.4 GHz after — any gap resets it):
  * PSUM split: tag "pv" (2 x 4KB slots: AV accumulators [65,1024], also
    v/proj psum) + tag "st" (2 x 4KB slots: score tiles, qk psum) so the
    next head-pair's score matmuls never wait on the previous pair's
    normalization.
  * Prologue: all 12 q/k projections in one ramped PE burst, interleaved
    with head-pair 0's score tiles (primes the ACT exp stream) and the v
    tiles (paced behind their DMA); weights stream in as row-halves in
    consumption order.
  * Steady state: per head pair only scores+AV remain; AV matmuls trail the
    score stream by 2 j-tiles so the pv-slot handoff has ~4us of slack and
    the PE phase-locks into a gapless [st,st,av,av] cadence at 2.4 GHz,
    paced by ACT's exp chain (~1.08us per [128,1024] tile).
  * Normalization: free the PSUM accumulators immediately (one bf16 copy
    per head incl. the colsum row), then entirely out of SBUF: sum rows
    regathered to [128,16] via direct SBUF->SBUF partition-scatter DMAs
    (lane-parallel ~0.25us reciprocal vs 6.5us single-lane), scattered to
    DRAM, broadcast back via zero-partition-stride DRAM APs, DVE multiply;
    odd head staged to its ovT partition range by an SBUF->SBUF DMA.
  * Projection in two passes: e=0..4 partials (4-deep across both PSUM
    pools) overlap the last head pair's normalization chain; the e=5 term,
    bias and chunked stores finish each i-tile.
"""
import numpy as np

NUM_HEADS = 12
E = 768
D = 64
B = 8
N = 1024
HALF = D // 2


def _ensure_axon_hooks():
    """The NTFF profile hook registry module may be missing in a fresh
    container; (re)create it so trace=True profiling degrades gracefully."""
    try:
        import antenv.axon_hooks  # noqa: F401
        return
    except ImportError:
        pass
    try:
        import antenv
        import os
        p = os.path.join(os.path.dirname(antenv.__file__), "axon_hooks.py")
        with open(p, "w") as f:
            f.write(
                "_hook = None\n\n"
                "def set_axon_ntff_profile_hook(hook):\n"
                "    global _hook\n    _hook = hook\n\n"
                "def get_axon_ntff_profile_hook():\n"
                "    return _hook\n")
    except Exception:
        pass


_ensure_axon_hooks()


# ---------------------------------------------------------------- host math
def _rope_matrix():
    """M[h, x, d]: rope(q)[x] = sum_d M[h, x, d] * q[d] (float64)."""
    inv_freq = 1.0 / (10000.0 ** (np.arange(0, D, 2, dtype=np.float64) / D))
    t = np.arange(NUM_HEADS, dtype=np.float64)
    emb = np.concatenate([t[:, None] * inv_freq[None, :]] * 2, axis=-1)  # [H, D]
    cos, sin = np.cos(emb), np.sin(emb)
    M = np.zeros((NUM_HEADS, D, D))
    for h in range(NUM_HEADS):
        for d in range(D):
            M[h, d, d] = cos[h, d]
            if d < HALF:
                M[h, d, d + HALF] = -sin[h, d]
            else:
                M[h, d, d - HALF] = sin[h, d]
    return M


def _prep_weights(w_qkv, b_qkv, w_proj, b_proj):
    w = w_qkv.astype(np.float64)
    b = b_qkv.astype(np.float64)
    M = _rope_matrix()
    scale = float(D) ** (-0.5)
    w_q = w[:, 0:E].reshape(E, NUM_HEADS, D)
    w_k = w[:, E:2 * E].reshape(E, NUM_HEADS, D)
    b_q = b[0:E].reshape(NUM_HEADS, D)
    b_k = b[E:2 * E].reshape(NUM_HEADS, D)
    w_q2 = np.einsum('ehd,hxd->ehx', w_q, M) * scale
    b_q2 = np.einsum('hd,hxd->hx', b_q, M) * scale
    w_k2 = np.einsum('ehd,hxd->ehx', w_k, M)
    b_k2 = np.einsum('hd,hxd->hx', b_k, M)
    w_qk = np.ascontiguousarray(
        np.concatenate([w_q2.reshape(E, E), w_k2.reshape(E, E)], axis=1),
        dtype=np.float32)                                     # [E, 2E]
    b_qk = np.concatenate([b_q2.reshape(E), b_k2.reshape(E)]).astype(np.float32)
    w_v = np.ascontiguousarray(w[:, 2 * E:3 * E], dtype=np.float32)
    b_out = (b[2 * E:3 * E] @ w_proj.astype(np.float64)
             + b_proj.astype(np.float64)).astype(np.float32)
    return w_qk, b_qk, w_v, b_out


# ---------------------------------------------------------------- waitfix
def _split_excess_waits(nc):
    """walrus in this container rejects >4 sync waits per instruction (and
    fewer on Drain/SP-NoOp paths). Split overflow waits onto preceding
    same-engine 1-wait NOPs — semantically identical (sequencer blocks in
    order)."""
    import concourse.mybir as mybir
    import bass_rust
    counter = [0]

    def make_nop(engine):
        counter[0] += 1
        nop = bass_rust.InstNoOp(name=f"I-waitfix-{counter[0]}", ins=[], outs=[])
        nop.engine = engine
        return nop

    for fn in nc.m.functions:
        for bb in fn.blocks:
            insts = bb.instructions
            out = []
            changed = False
            for inst in insts:
                si = inst.sync_info
                waits = list(si.on_wait) if si is not None else []
                tn = type(inst).__name__
                keep = 0 if tn == "InstDrain" else 1
                if len(waits) > keep:
                    for w in waits[:len(waits) - keep]:
                        nop = make_nop(inst.engine)
                        nop.sync_info = mybir.SyncInfo(on_wait=[w], on_update=[])
                        out.append(nop)
                    inst.sync_info = mybir.SyncInfo(
                        on_wait=waits[len(waits) - keep:],
                        on_update=list(si.on_update))
                    changed = True
                out.append(inst)
            if changed:
                bb.instructions = out


# ---------------------------------------------------------------- device IR
_NC_CACHE = []


def _build_nc(waitfix=True):
    import concourse.bass as bass
    import concourse.mybir as mybir
    from concourse.tile import TileContext

    dt = mybir.dt
    f32 = dt.float32
    bf16 = dt.bfloat16
    AF = mybir.ActivationFunctionType

    nc = bass.Bass(target_bir_lowering=False)
    f8 = dt.float8e4
    xT_d = nc.dram_tensor("xT", [E, N], bf16, kind="ExternalInput")
    xT8_d = nc.dram_tensor("xT8", [3, 128, 2 * N], f8, kind="ExternalInput")
    wqk8_d = nc.dram_tensor("w_qk8", [3, 128, 4 * E], f8, kind="ExternalInput")
    bqk_d = nc.dram_tensor("b_qk", [2 * E], f32, kind="ExternalInput")
    wv_d = nc.dram_tensor("w_v", [E, E], bf16, kind="ExternalInput")
    wp_d = nc.dram_tensor("w_proj", [E, E], bf16, kind="ExternalInput")
    bo_d = nc.dram_tensor("b_out", [E], f32, kind="ExternalInput")
    y_d = nc.dram_tensor("y", [N, E], f32, kind="ExternalOutput")

    ET = E // 128          # 6 e-tiles
    IT = N // 128          # 8 i/j-tiles
    HP = NUM_HEADS // 2    # 6 head pairs

    with TileContext(nc) as tc:
        with (
            tc.tile_pool(name="load", bufs=1) as pl,
            tc.tile_pool(name="qtk", bufs=12) as pqt,
            tc.tile_pool(name="pt", bufs=16) as ppt,
            tc.tile_pool(name="vaug", bufs=1) as pva,
            tc.tile_pool(name="ov", bufs=1) as pov,
            tc.tile_pool(name="raw", bufs=4) as prw,
            tc.tile_pool(name="rs", bufs=2) as prs,
            tc.tile_pool(name="rb", bufs=4) as prb,
            tc.tile_pool(name="tmp", bufs=2) as ptm,
            tc.tile_pool(name="yst", bufs=2) as pys,
            tc.tile_pool(name="psv", bufs=2, space="PSUM") as psv,
            tc.tile_pool(name="psst", bufs=2, space="PSUM") as pss,
            tc.tile_pool(name="dscr", bufs=4, space="DRAM") as pdram,
        ):
            # ---- input loads (xT/wqk first: they gate the first PE work)
            xT = [pl.tile([128, N], bf16, tag=f"xT{e}", name=f"xT{e}")
                  for e in range(ET)]
            xT8 = [pl.tile([128, 2 * N], f8, tag=f"xT8_{p}", name=f"xT8_{p}")
                   for p in range(3)]
            wqk8 = [pl.tile([128, 4 * E], f8, tag=f"wqk8_{p}",
                            name=f"wqk8_{p}") for p in range(3)]
            wv = [pl.tile([128, E], bf16, tag=f"wv{e}", name=f"wv{e}")
                  for e in range(ET)]
            wp = [pl.tile([128, E], bf16, tag=f"wp{e}", name=f"wp{e}")
                  for e in range(ET)]
            # all weight/activation loads as partition-row halves: full-width
            # lines (good DMA efficiency) on 2x the queues, in consumption
            # order (xT+wqk gate the first matmuls, wv the v phase, wp proj)
            def load_halves(tiles, dram, e):
                for h in range(2):
                    rs_ = slice(e * 128 + h * 64, e * 128 + (h + 1) * 64)
                    nc.sync.dma_start(out=tiles[e][h * 64:(h + 1) * 64, :],
                                      in_=dram[rs_, :])
            # fp8 qk operands first (they gate the first matmuls), then the
            # bf16 xT (v path); all as row-halves across queues
            for p in range(3):
                for h in range(2):
                    rs_ = slice(h * 64, (h + 1) * 64)
                    nc.sync.dma_start(out=wqk8[p][rs_, :],
                                      in_=wqk8_d[p, rs_, :])
                    nc.sync.dma_start(out=xT8[p][rs_, :],
                                      in_=xT8_d[p, rs_, :])
            for e in range(ET):
                load_halves(xT, xT_d, e)
            bq = pl.tile([128, 12], f32, tag="bq")
            nc.sync.dma_start(out=bq, in_=bqk_d[:].rearrange("(t p) -> p t", p=128))
            for e in range(ET):
                load_halves(wv, wv_d, e)
            for e in range(ET):
                load_halves(wp, wp_d, e)
            bo = pl.tile([128, E], f32, tag="bo")
            nc.sync.dma_start(
                out=bo,
                in_=bass.AP(tensor=bo_d[:].tensor, offset=bo_d[:].offset,
                            ap=[[0, 128], [1, E]]))

            v_aug = [pva.tile([128, NUM_HEADS * (D + 1)], bf16, tag=f"vaug{i}",
                              name=f"vaug{i}") for i in range(IT)]
            ovT = [pov.tile([128, N], bf16, tag=f"ovT{e}", name=f"ovT{e}")
                   for e in range(ET)]

            # ---------------- emit helpers
            def emit_qk(ct):
                """q^T (or k^T) for column-tile ct -> [128, N] bf16 SBUF.

                fp8 DoubleRow: each matmul contracts a 256-row e-pair (two
                k-subtiles along the free dim), 0.5 cycles/row. Values are
                scaled by 256 (folded into weights host-side; exp applies
                1/65536), keeping fp8 quantization in the e4m3 normal range.
                """
                pq = pss.tile([128, N], f32, tag="st", name=f"pq{ct}")
                for ih in range(2):
                    for p in range(3):
                        wv3 = wqk8[p].rearrange("q (t c) -> q t c", t=2)
                        xv3 = xT8[p].rearrange("q (t c) -> q t c", t=2)
                        nc.tensor.matmul(
                            pq[:, ih * 512:(ih + 1) * 512],
                            wv3[:, :, ct * 128:(ct + 1) * 128],
                            xv3[:, :, ih * 512:(ih + 1) * 512],
                            start=(p == 0), stop=(p == 2),
                            perf_mode=mybir.MatmulPerfMode.DoubleRow)
                dst = pqt.tile([128, N], bf16, tag="qt", name=f"qkT{ct}")
                nc.vector.tensor_scalar_add(dst, pq, bq[:, ct:ct + 1])
                return dst

            def emit_v(it):
                """v rows for i-tile it -> v_aug[it] (bf16, ones cols)."""
                pv = psv.tile([128, E], f32, tag="pv", name=f"pv_{it}")
                for (n0, nw) in ((0, 512), (512, 256)):
                    for e in range(ET):
                        nc.tensor.matmul(
                            pv[:, n0:n0 + nw],
                            xT[e][:, it * 128:(it + 1) * 128],
                            wv[e][:, n0:n0 + nw],
                            start=(e == 0), stop=(e == ET - 1))
                va = v_aug[it]
                out_v = va.rearrange("p (h c) -> p h c", c=65)[:, :, 0:64]
                in_v = pv.rearrange("p (h c) -> p h c", c=64)
                nc.vector.tensor_copy(out=out_v, in_=in_v)
                ones_cols = va.rearrange("p (h c) -> p h c", c=65)[:, :, 64:65]
                bq12 = bq[:, 0:12].rearrange("p (a b) -> p a b", b=1)
                nc.vector.tensor_scalar(
                    ones_cols, bq12, 0.0, 1.0,
                    mybir.AluOpType.mult, mybir.AluOpType.add)

            def emit_st(hp, jt, h, qt, kt):
                """scores^T tile for (head-pair hp, j-tile jt, head h) + exp."""
                js = slice(jt * 128, (jt + 1) * 128)
                hs = slice(h * 64, h * 64 + 64)
                st = pss.tile([128, N], f32, tag="st", name=f"st{hp}_{jt}_{h}")
                for ih in range(2):
                    isl = slice(ih * 512, (ih + 1) * 512)
                    nc.tensor.matmul(st[:, isl], kt[hs, js], qt[hs, isl])
                pt = ppt.tile([128, N], bf16, tag="pT", name=f"pT{hp}_{jt}_{h}")
                nc.scalar.activation(out=pt, in_=st, func=AF.Exp,
                                     scale=1.0 / 65536.0)
                return pt

            def emit_av(hp, jt, pts, pv0, pv1):
                h0, h1 = 2 * hp, 2 * hp + 1
                pt0, pt1 = pts
                for (pv, hh, pt) in ((pv0, h0, pt0), (pv1, h1, pt1)):
                    for ih in range(2):
                        isl = slice(ih * 512, (ih + 1) * 512)
                        nc.tensor.matmul(
                            pv[:, isl], v_aug[jt][:, hh * 65:hh * 65 + 65],
                            pt[:, isl], start=(jt == 0), stop=(jt == IT - 1))

            def emit_norm(hp, pv0, pv1):
                """ovT[hp] = pv / colsums.

                pv PSUM slots gate the NEXT head-pair's AV matmuls, so free
                them ASAP: one bf16 copy per head to SBUF (raw out + sums),
                then normalize entirely out of SBUF. The [1,N] colsum rows
                are single-lane; a direct DVE reciprocal is 6-pass (~6.5us),
                so round-trip through DRAM to regather them as [128,16]
                (lane-parallel, ~0.25us), scatter back, broadcast-in via
                zero-partition-stride DRAM APs, DVE multiply."""
                raw0 = prw.tile([65, N], bf16, tag="raw", name=f"raw0_{hp}")
                raw1 = prw.tile([65, N], bf16, tag="raw", name=f"raw1_{hp}")
                nc.vector.tensor_copy(out=raw0, in_=pv0)
                if hp == HP - 1:
                    # last pair: ACT is idle; run the second copy there in
                    # parallel so the projection tail starts sooner
                    nc.scalar.activation(out=raw1, in_=pv1, func=AF.Copy)
                else:
                    nc.vector.tensor_copy(out=raw1, in_=pv1)
                # lane-parallel reciprocal: gather both [1,N] sum rows into
                # [128,16] via direct SBUF->SBUF partition-scatter DMAs
                rg = prs.tile([128, 16], bf16, tag="rg", name=f"rg{hp}")
                nc.sync.dma_start(
                    out=rg[0:64, :],
                    in_=raw0[64:65, :].rearrange("p (a b) -> p a b", b=16))
                nc.sync.dma_start(
                    out=rg[64:128, :],
                    in_=raw1[64:65, :].rearrange("p (a b) -> p a b", b=16))
                rgr = prs.tile([128, 16], bf16, tag="rgr", name=f"rgr{hp}")
                with nc.allow_low_precision(reason="softmax denominators"):
                    nc.vector.reciprocal(out=rgr, in_=rg)
                dr = pdram.tile([1, 2 * N], bf16, tag="dr", name=f"dr{hp}")
                nc.sync.dma_start(
                    out=bass.AP(tensor=dr.tensor, offset=dr.offset,
                                ap=[[16, 128], [1, 16]]),
                    in_=rgr)
                rb0 = prb.tile([64, N], bf16, tag="rb", name=f"rb0_{hp}")
                rb1 = prb.tile([64, N], bf16, tag="rb", name=f"rb1_{hp}")
                nc.sync.dma_start(
                    out=rb0, in_=bass.AP(tensor=dr.tensor, offset=dr.offset,
                                         ap=[[0, 64], [1, N]]))
                nc.sync.dma_start(
                    out=rb1, in_=bass.AP(tensor=dr.tensor,
                                         offset=dr.offset + N,
                                         ap=[[0, 64], [1, N]]))
                tmp1 = ptm.tile([64, N], bf16, tag="tmp", name=f"tmp{hp}")
                for c in (slice(0, 512), slice(512, N)):
                    nc.vector.tensor_mul(ovT[hp][0:64, c], raw0[0:64, c],
                                         rb0[:, c])
                    nc.vector.tensor_mul(tmp1[:, c], raw1[0:64, c],
                                         rb1[:, c])
                    nc.sync.dma_start(out=ovT[hp][64:128, c], in_=tmp1[:, c])

            # ---------------- prologue: all 12 qk projections in one ramped
            # PE burst, with hp0's score tiles interleaved to prime ACT
            qts, kts = {}, {}
            pts = {}

            def emit_qk_pair(ct_a, ct_b):
                """First q/k pair with e-interleaved accumulation: matmuls
                track the (wqk[e], xT[e]) DMA arrival order, halving the
                load-paced stalls at kernel start."""
                pqa = pss.tile([128, N], f32, tag="st", name=f"pq{ct_a}")
                pqb = pss.tile([128, N], f32, tag="st", name=f"pq{ct_b}")
                for p in range(3):
                    wv3 = wqk8[p].rearrange("q (t c) -> q t c", t=2)
                    xv3 = xT8[p].rearrange("q (t c) -> q t c", t=2)
                    for (pq, ct) in ((pqa, ct_a), (pqb, ct_b)):
                        for ih in range(2):
                            nc.tensor.matmul(
                                pq[:, ih * 512:(ih + 1) * 512],
                                wv3[:, :, ct * 128:(ct + 1) * 128],
                                xv3[:, :, ih * 512:(ih + 1) * 512],
                                start=(p == 0), stop=(p == 2),
                                perf_mode=mybir.MatmulPerfMode.DoubleRow)
                outs = []
                for (pq, ct) in ((pqa, ct_a), (pqb, ct_b)):
                    dst = pqt.tile([128, N], bf16, tag="qt", name=f"qkT{ct}")
                    nc.vector.tensor_scalar_add(dst, pq, bq[:, ct:ct + 1])
                    outs.append(dst)
                return outs

            for hp0 in range(HP):
                if hp0 == 0:
                    qts[0], kts[0] = emit_qk_pair(0, HP)
                    continue
                qts[hp0] = emit_qk(hp0)
                kts[hp0] = emit_qk(HP + hp0)
                if hp0 >= 1:
                    jt = hp0 - 1
                    pts[(jt, 0)] = emit_st(0, jt, 0, qts[0], kts[0])
                    pts[(jt, 1)] = emit_st(0, jt, 1, qts[0], kts[0])
                if 2 <= hp0 <= 5:
                    emit_v(2 * (hp0 - 2))
                    emit_v(2 * (hp0 - 2) + 1)
            # hp0's remaining score tiles + v tail + AV catch-up
            pv0 = psv.tile([65, N], f32, tag="pv", name="pv0_0")
            pv1 = psv.tile([65, N], f32, tag="pv", name="pv1_0")
            pts[(5, 0)] = emit_st(0, 5, 0, qts[0], kts[0])
            pts[(5, 1)] = emit_st(0, 5, 1, qts[0], kts[0])
            emit_av(0, 0, (pts.pop((0, 0)), pts.pop((0, 1))), pv0, pv1)
            emit_av(0, 1, (pts.pop((1, 0)), pts.pop((1, 1))), pv0, pv1)
            pts[(6, 0)] = emit_st(0, 6, 0, qts[0], kts[0])
            pts[(6, 1)] = emit_st(0, 6, 1, qts[0], kts[0])
            emit_av(0, 2, (pts.pop((2, 0)), pts.pop((2, 1))), pv0, pv1)
            emit_av(0, 3, (pts.pop((3, 0)), pts.pop((3, 1))), pv0, pv1)
            pts[(7, 0)] = emit_st(0, 7, 0, qts[0], kts[0])
            pts[(7, 1)] = emit_st(0, 7, 1, qts[0], kts[0])
            for jt in range(4, IT):
                emit_av(0, jt, (pts.pop((jt, 0)), pts.pop((jt, 1))), pv0, pv1)
            emit_norm(0, pv0, pv1)

            # ---------------- steady-state head pairs 1..5 (scores + AV only)
            # AVs trail the score stream by 2 j-tiles: the pv-slot handoff at
            # each head-pair boundary (previous pair's raw copies) then has
            # ~4us of slack, so the PE phase-locks into the gapless
            # [st,st,av,av] pattern instead of the 1.2GHz fragmented mode
            for hp in range(1, HP):
                qt, kt = qts[hp], kts[hp]
                pv0 = psv.tile([65, N], f32, tag="pv", name=f"pv0_{hp}")
                pv1 = psv.tile([65, N], f32, tag="pv", name=f"pv1_{hp}")
                lag = {}
                for jt in range(IT):
                    if jt >= 2:
                        emit_av(hp, jt - 2, lag.pop(jt - 2), pv0, pv1)
                    lag[jt] = (emit_st(hp, jt, 0, qt, kt),
                               emit_st(hp, jt, 1, qt, kt))
                for jt in range(IT - 2, IT):
                    emit_av(hp, jt, lag.pop(jt), pv0, pv1)
                emit_norm(hp, pv0, pv1)

            # ---------------- projection: y = ovT^T @ w_proj + b_out
            # two passes: e=0..4 partials (ready after hp4) keep the PE busy
            # while hp5's normalization chain completes; e=5 finishes each
            pyts = {}

            def emit_proj_partial(it):
                pool, tag = (psv, "pv") if it % 2 == 0 else (pss, "st")
                isl = slice(it * 128, (it + 1) * 128)
                pyt = pool.tile([128, E], f32, tag=tag, name=f"py_{it}")
                for (n0, nw) in ((0, 512), (512, 256)):
                    for e in range(ET - 1):
                        nc.tensor.matmul(
                            pyt[:, n0:n0 + nw],
                            ovT[e][:, isl],
                            wp[e][:, n0:n0 + nw],
                            start=(e == 0), stop=False)
                pyts[it] = pyt

            def emit_proj_finish(it):
                isl = slice(it * 128, (it + 1) * 128)
                pyt = pyts.pop(it)
                for (n0, nw) in ((0, 512), (512, 256)):
                    nc.tensor.matmul(
                        pyt[:, n0:n0 + nw],
                        ovT[ET - 1][:, isl],
                        wp[ET - 1][:, n0:n0 + nw],
                        start=False, stop=True)
                ysb = pys.tile([128, E], f32, tag="y", name=f"y{it}")
                nc.vector.tensor_add(ysb, pyt, bo)
                nc.sync.dma_start(out=y_d[isl, 0:512], in_=ysb[:, 0:512])
                nc.sync.dma_start(out=y_d[isl, 512:E], in_=ysb[:, 512:E])

            for it in range(4):
                emit_proj_partial(it)
            for it in range(4):
                emit_proj_finish(it)
                emit_proj_partial(it + 4)
            for it in range(4, IT):
                emit_proj_finish(it)

    if waitfix:
        _split_excess_waits(nc)
    return nc


def _get_nc():
    if not _NC_CACHE:
        _NC_CACHE.append(_build_nc())
    return _NC_CACHE[0]


# ---------------------------------------------------------------- entry point
def kernel(x, w_qkv, b_qkv, w_proj, b_proj, _trace=False):
    from concourse.bass_utils import run_bass_kernel_spmd

    import ml_dtypes
    bf16 = ml_dtypes.bfloat16
    x = np.asarray(x)
    w_qk, b_qk, w_v, b_out = _prep_weights(
        np.asarray(w_qkv), np.asarray(b_qkv), np.asarray(w_proj),
        np.asarray(b_proj))
    f8 = ml_dtypes.float8_e4m3
    SW = 256.0

    def to_f8(a):
        return np.clip(a, -240.0, 240.0).astype(f8)

    def dr_pairs(a, free):
        # [E, free] -> [3, 128, 2*free] with the two 128-row k-subtiles of
        # each 256-row pair concatenated along the free dim (DoubleRow lhs/rhs)
        return np.ascontiguousarray(
            a.reshape(3, 2, 128, free).transpose(0, 2, 1, 3).reshape(
                3, 128, 2 * free))

    w_qk8 = dr_pairs(to_f8(w_qk * SW), 2 * E)
    w_v16 = w_v.astype(bf16)
    w_proj16 = np.ascontiguousarray(np.asarray(w_proj)).astype(bf16)

    in_maps = []
    for b in range(B):
        xTb = np.ascontiguousarray(x[b].T)
        in_maps.append({
            "xT": xTb.astype(bf16),
            "xT8": dr_pairs(to_f8(xTb), N),
            "w_qk8": w_qk8,
            "b_qk": b_qk * np.float32(SW),
            "w_v": w_v16,
            "w_proj": w_proj16,
            "b_out": b_out,
        })

    nc = _get_nc()
    res = run_bass_kernel_spmd(nc, in_maps, core_ids=list(range(B)),
                               trace=_trace)
    out = np.stack([res.results[b]["y"] for b in range(B)]).astype(np.float32)
    if _trace:
        return out, res
    return out
